# revision 21
# baseline (speedup 1.0000x reference)
"""Trainium2 Bass kernel for nn_AttentionModule (sparse_attention).

Computation (per batch b):
  qe = LN(MLP(q[b]))            (16,)
  ke = LN(MLP(k[b,:,0,:].T))    (4, 16)
  logits = qe @ ke.T * DIM^-0.5 - modality_dropout*1e5
  attn = softmax(logits / 10)   (4,)
  out[b, vc, p] = sum_c attn[c] * v[b, vc, p, c]
  attn_map = broadcast(attn)

Sharding: pure data parallel over batch across 8 NeuronCores (8 batches each).
MLP/LN params replicated. The heavy part (streaming 4 MB of v per batch) is
memory bound: per core 32 MB in + 8 MB out ~= 111 us at ~360 GB/s.

Device layout per batch: v[b] viewed flat as (128, 7840) f32 (partition-
contiguous DMA); channel c lives at free-dim stride 4. Weighted sum =
4 ScalarE multiplies (per-partition scale = attn weight broadcast via a
tiny TensorE outer-product) + 3 VectorE adds; result (128, 1960) DMAs out
contiguously. attn_map is a pure broadcast of the (64,4) attn tensor, so it
is materialized host-side from the device-computed attn.
"""

import numpy as np
from contextlib import ExitStack

import concourse.bass as bass
import concourse.tile as tile
from concourse import mybir
from concourse.bass_utils import run_bass_kernel_spmd

N_CORES = 8
B = 64
BPC = B // N_CORES  # batches per core
V_CH = 5
P = 50176
C = 4
DIM = 10
IMG = 224
ROWS = 128
FW = V_CH * P * C // ROWS   # 7840 f32 per partition per batch (with channels)
FO = V_CH * P // ROWS       # 1960 outputs per partition per batch
SCALE_T = float(DIM ** -0.5) / 10.0   # attn scale folded with temperature
MD_SCALE = 1e5 / 10.0
LN_EPS = 1e-5
F32 = mybir.dt.float32
AL = mybir.AluOpType
AF = mybir.ActivationFunctionType
AX = mybir.AxisListType


def _cap_sync_waits(nc, max_waits=1):
    """walrus in this container rejects >1 sync-wait per instruction ("Too
    many sync wait commands"). Spill excess waits onto same-engine nops
    inserted immediately before the instruction (engine streams execute bb
    instructions in order, so the waits still complete first)."""

    def _pop_by_name(name):
        for f2 in nc.m.functions:
            for b2 in f2.blocks:
                lst = b2.instructions
                for j in range(len(lst) - 1, -1, -1):
                    if lst[j].name == name:
                        return lst.pop(j)
        raise RuntimeError(f"spill nop {name} not found")

    for fn in nc.m.functions:
        for blk in fn.blocks:
            insts = blk.instructions
            i = 0
            while i < len(insts):
                ins = insts[i]
                si = ins.sync_info
                waits = list(si.on_wait) if si is not None and si.on_wait else []
                keep = 0 if type(ins).__name__ == "InstDrain" else max_waits
                if len(waits) > keep:
                    upds = (
                        list(si.on_update)
                        if si is not None and si.on_update
                        else []
                    )
                    spill = waits[keep:]
                    ins.sync_info = mybir.SyncInfo(
                        on_wait=waits[:keep], on_update=upds
                    )
                    for w in spill:
                        bi = nc.engines[ins.engine].nop(
                            nofuse=True, hint="wait_spill"
                        )
                        nop_inst = _pop_by_name(bi.ins.name)
                        nop_inst.sync_info = mybir.SyncInfo(
                            on_wait=[w], on_update=[]
                        )
                        insts.insert(i, nop_inst)
                        i += 1
                i += 1


PARAM_LAYOUT = [
    # (name, row_off, rows, cols) packed into one (128, PARAM_K) array
    ("xq", 0, DIM, BPC),
    ("xk", 0, DIM, C * BPC),
    ("mdp", 0, BPC, C),          # modality_dropout * 1e5/T, prescaled on host
    ("qw1", 0, DIM, 128),
    ("kw1", 0, DIM, 128),
    ("qw2", 0, 128, 16),
    ("kw2", 0, 128, 16),
    ("b1c", 0, 128, 40),         # [qb1 x8 | kb1 x32] per-column layer-1 bias
    ("b2c", 0, 40, 16),          # rows 0:32 kb2, rows 32:40 qb2
    ("gc", 0, 40, 16),           # rows 0:32 kg, rows 32:40 qg*SCALE_T
    ("bec", 0, 40, 16),          # rows 0:32 kbeta, rows 32:40 qbeta*SCALE_T
    ("eye", 0, 32, 32),
    ("eye8h", 32, BPC, BPC),     # identity living at partitions 32:40
    ("mask", 0, BPC, C * BPC),
    ("mask4", 0, C * BPC, C),
    ("ones1", 0, 1, 128),
    ("eps", 0, 40, 1),
]
PARAM_OFF = {}
_off = 0
for _n, _ro, _r, _c in PARAM_LAYOUT:
    PARAM_OFF[_n] = _off
    _off += _c
PARAM_K = _off


def _build():
    nc = bass.Bass("TRN2", target_bir_lowering=False, debug=False)

    v_d = nc.dram_tensor("v", [BPC * ROWS, FW], F32, kind="ExternalInput")
    params_d = nc.dram_tensor("params", [128, PARAM_K], F32, kind="ExternalInput")

    out_d = nc.dram_tensor("out", [BPC * ROWS, FO], F32, kind="ExternalOutput")
    attn_d = nc.dram_tensor("attn", [BPC, C], F32, kind="ExternalOutput")

    with tile.TileContext(nc) as tc, ExitStack() as ctx:
        singles = ctx.enter_context(tc.tile_pool(name="singles", bufs=1))
        small = ctx.enter_context(tc.tile_pool(name="small", bufs=1))
        psum = ctx.enter_context(tc.tile_pool(name="psum", bufs=1, space="PSUM"))
        vpool = ctx.enter_context(tc.tile_pool(name="vpool", bufs=4))
        tpool = ctx.enter_context(tc.tile_pool(name="tpool", bufs=2))

        params = singles.tile([128, PARAM_K], F32, tag="params")
        nc.sync.dma_start(out=params, in_=params_d.ap())

        def pv(name):
            for n, ro, r, c in PARAM_LAYOUT:
                if n == name:
                    off = PARAM_OFF[name]
                    return params[ro : ro + r, off : off + c]
            raise KeyError(name)

        xq, xk, mdp = pv("xq"), pv("xk"), pv("mdp")
        qw1, kw1, qw2, kw2 = pv("qw1"), pv("kw1"), pv("qw2"), pv("kw2")
        b1c, b2c, gc, bec = pv("b1c"), pv("b2c"), pv("gc"), pv("bec")
        eye, eye8h = pv("eye"), pv("eye8h")
        mask, mask4, ones1, eps_t = pv("mask"), pv("mask4"), pv("ones1"), pv("eps")

        # ---- fused q+k MLP: layer 1 on a combined (128, 40) tile
        # free cols 0:8 = q batches, 8:40 = k (b,c) pairs
        h1ps = psum.tile([128, BPC + C * BPC], F32, tag="psA")
        nc.tensor.matmul(out=h1ps[:, 0:BPC], lhsT=qw1, rhs=xq)
        nc.tensor.matmul(out=h1ps[:, BPC : BPC + C * BPC], lhsT=kw1, rhs=xk)
        # bias + LeakyReLU(0.1) by hand (the ACT Lrelu table bakes alpha=0.01)
        h1 = small.tile([128, BPC + C * BPC], F32, tag="h1")
        h1t = small.tile([128, BPC + C * BPC], F32, tag="h1t")
        nc.vector.tensor_tensor(out=h1, in0=h1ps, in1=b1c, op=AL.add)
        nc.vector.tensor_scalar(
            out=h1t, in0=h1, scalar1=0.1, scalar2=None, op0=AL.mult
        )
        nc.vector.tensor_tensor(out=h1, in0=h1, in1=h1t, op=AL.max)

        # ---- layer 2 into a combined (40, 16) tile: rows 0:32 ke, 32:40 qe
        lnps = psum.tile([C * BPC + BPC, 16], F32, tag="psB")
        nc.tensor.matmul(
            out=lnps[C * BPC : C * BPC + BPC, :], lhsT=h1[:, 0:BPC], rhs=qw2
        )
        nc.tensor.matmul(
            out=lnps[0 : C * BPC, :], lhsT=h1[:, BPC : BPC + C * BPC], rhs=kw2
        )
        hh = small.tile([40, 16], F32, tag="hh")
        nc.vector.tensor_tensor(out=hh, in0=lnps, in1=b2c, op=AL.add)
        # one LayerNorm over both paths; q rows get gamma/beta pre-scaled by
        # DIM^-0.5/TEMPERATURE so the logits come out ready for softmax
        stats = small.tile([40, nc.vector.BN_STATS_DIM], F32, tag="st")
        nc.vector.bn_stats(out=stats, in_=hh)
        mv = small.tile([40, nc.vector.BN_AGGR_DIM], F32, tag="mv")
        nc.vector.bn_aggr(out=mv, in_=stats)
        nc.vector.tensor_scalar(
            out=hh, in0=hh, scalar1=mv[:, 0:1], scalar2=None, op0=AL.subtract
        )
        std = small.tile([40, 1], F32, tag="std")
        nc.scalar.activation(out=std, in_=mv[:, 1:2], func=AF.Sqrt, bias=eps_t)
        rstd = small.tile([40, 1], F32, tag="rstd")
        nc.vector.reciprocal(out=rstd, in_=std)
        nc.vector.tensor_scalar(
            out=hh, in0=hh, scalar1=rstd, scalar2=None, op0=AL.mult
        )
        nc.vector.tensor_tensor(out=hh, in0=hh, in1=gc, op=AL.mult)
        nc.vector.tensor_tensor(out=hh, in0=hh, in1=bec, op=AL.add)

        # ---- logits L2[b', 4b+c] = qe'[b'] . ke[4b+c]; keep diagonal b'=b
        qeT_ps = psum.tile([16, BPC], F32, tag="psC")
        nc.tensor.transpose(
            out=qeT_ps, in_=hh[C * BPC : C * BPC + BPC, :], identity=eye8h
        )
        qeT = small.tile([16, BPC], F32, tag="qeT_sb")
        nc.vector.tensor_copy(out=qeT, in_=qeT_ps)
        keT_ps = psum.tile([16, C * BPC], F32, tag="psD")
        nc.tensor.transpose(out=keT_ps, in_=hh[0 : C * BPC, :], identity=eye)
        keT = small.tile([16, C * BPC], F32, tag="keT_sb")
        nc.vector.tensor_copy(out=keT, in_=keT_ps)
        l2_ps = psum.tile([BPC, C * BPC], F32, tag="psB")
        nc.tensor.matmul(out=l2_ps, lhsT=qeT, rhs=keT)
        zm = small.tile([BPC, C * BPC], F32, tag="zm")
        nc.vector.tensor_tensor(out=zm, in0=l2_ps, in1=mask, op=AL.mult)
        z = small.tile([BPC, C], F32, tag="z")
        nc.vector.tensor_reduce(
            out=z,
            in_=zm.rearrange("p (b2 c) -> p c b2", c=C),
            axis=AX.X,
            op=AL.add,
        )

        # ---- masked softmax over the 4 modalities (z already /T-scaled)
        nc.vector.tensor_tensor(out=z, in0=z, in1=mdp, op=AL.subtract)
        rmax = small.tile([BPC, 1], F32, tag="rmax")
        nc.vector.tensor_reduce(out=rmax, in_=z, axis=AX.X, op=AL.max)
        nmax = small.tile([BPC, 1], F32, tag="nmax")
        nc.vector.tensor_scalar(
            out=nmax, in0=rmax, scalar1=-1.0, scalar2=None, op0=AL.mult
        )
        e = small.tile([BPC, C], F32, tag="e")
        nc.scalar.activation(out=e, in_=z, func=AF.Exp, bias=nmax, scale=1.0)
        ssum = small.tile([BPC, 1], F32, tag="ssum")
        nc.vector.tensor_reduce(out=ssum, in_=e, axis=AX.X, op=AL.add)
        rs = small.tile([BPC, 1], F32, tag="rs")
        nc.vector.reciprocal(out=rs, in_=ssum)
        attn_sb = small.tile([BPC, C], F32, tag="attn_sb")
        nc.vector.tensor_scalar(
            out=attn_sb, in0=e, scalar1=rs, scalar2=None, op0=AL.mult
        )
        nc.sync.dma_start(out=attn_d.ap(), in_=attn_sb)

        # ---- broadcast attn to all 128 partitions: wall[p, 4b+c] = attn[b, c]
        o32_ps = psum.tile([C * BPC, C], F32, tag="psB")
        nc.tensor.matmul(out=o32_ps, lhsT=mask, rhs=attn_sb)
        o32 = small.tile([C * BPC, C], F32, tag="o32_sb")
        nc.vector.tensor_tensor(out=o32, in0=o32_ps, in1=mask4, op=AL.mult)
        attn32 = small.tile([C * BPC, 1], F32, tag="attn32")
        nc.vector.tensor_reduce(out=attn32, in_=o32, axis=AX.X, op=AL.add)
        arow_ps = psum.tile([1, C * BPC], F32, tag="psC")
        nc.tensor.transpose(out=arow_ps, in_=attn32, identity=eye)
        arow = small.tile([1, C * BPC], F32, tag="arow_sb")
        nc.vector.tensor_copy(out=arow, in_=arow_ps)
        wps = psum.tile([128, C * BPC], F32, tag="psD")
        nc.tensor.matmul(out=wps, lhsT=ones1, rhs=arow)
        wall = singles.tile([128, C * BPC], F32, tag="wall_sb")
        nc.vector.tensor_copy(out=wall, in_=wps)

        # ---- big weighted sum over v, one batch at a time
        v_ap = v_d.ap()
        o_ap = out_d.ap()
        for b in range(BPC):
            T = vpool.tile([ROWS, FW], F32, tag="T")
            nc.sync.dma_start(out=T, in_=v_ap[b * ROWS : (b + 1) * ROWS, :])
            Tv = T.rearrange("p (f c) -> p c f", c=C)  # channel slice = stride 4
            tts = []
            for cc in range(C):
                tcc = tpool.tile([ROWS, FO], F32, tag=f"t{cc}")
                w_ap = wall[:, C * b + cc : C * b + cc + 1]
                ch = Tv[:, cc : cc + 1, :].squeeze(1)
                if cc % 2 == 0:
                    nc.scalar.mul(out=tcc, in_=ch, mul=w_ap)
                else:
                    nc.vector.tensor_scalar(
                        out=tcc, in0=ch, scalar1=w_ap, scalar2=None, op0=AL.mult
                    )
                tts.append(tcc)
            nc.vector.tensor_tensor(out=tts[0], in0=tts[0], in1=tts[1], op=AL.add)
            nc.vector.tensor_tensor(out=tts[2], in0=tts[2], in1=tts[3], op=AL.add)
            nc.vector.tensor_tensor(out=tts[0], in0=tts[0], in1=tts[2], op=AL.add)
            # out-DMA via the POOL SWDGE queue stream: keeps the SP HWDGE
            # ring (FIFO) exclusively feeding 4MB v-input transfers.
            nc.gpsimd.dma_start(out=o_ap[b * ROWS : (b + 1) * ROWS, :], in_=tts[0])

    _cap_sync_waits(nc)
    return nc


_CACHE = {}


def _get_nc():
    if "nc" not in _CACHE:
        _CACHE["nc"] = _build()
    return _CACHE["nc"]


def _pack_params(blocks):
    """Pack named blocks into one (128, PARAM_K) f32 array per layout."""
    A = np.zeros((128, PARAM_K), np.float32)
    for name, ro, rows, cols in PARAM_LAYOUT:
        blk = blocks[name]
        assert blk.shape == (rows, cols), (name, blk.shape, rows, cols)
        off = PARAM_OFF[name]
        A[ro : ro + rows, off : off + cols] = blk
    return A


def _make_in_maps(q, k, v, modality_dropout,
                  q_w1, q_b1, q_w2, q_b2, q_g, q_beta,
                  k_w1, k_b1, k_w2, k_b2, k_g, k_beta):
    f = np.float32
    q = np.asarray(q, dtype=f)
    k = np.asarray(k, dtype=f)
    v = np.asarray(v, dtype=f)
    md = np.asarray(modality_dropout, dtype=f)
    b1c = np.empty((128, 40), f)
    b1c[:, 0:BPC] = np.asarray(q_b1, f)[:, None]
    b1c[:, BPC:] = np.asarray(k_b1, f)[:, None]
    b2c = np.concatenate(
        [np.tile(np.asarray(k_b2, f), (32, 1)), np.tile(np.asarray(q_b2, f), (BPC, 1))]
    )
    gc = np.concatenate(
        [np.tile(np.asarray(k_g, f), (32, 1)),
         np.tile(np.asarray(q_g, f) * np.float32(SCALE_T), (BPC, 1))]
    )
    bec = np.concatenate(
        [np.tile(np.asarray(k_beta, f), (32, 1)),
         np.tile(np.asarray(q_beta, f) * np.float32(SCALE_T), (BPC, 1))]
    )
    const_blocks = {
        "qw1": np.asarray(q_w1, f).T,
        "kw1": np.asarray(k_w1, f).T,
        "qw2": np.asarray(q_w2, f).T,
        "kw2": np.asarray(k_w2, f).T,
        "b1c": b1c,
        "b2c": b2c,
        "gc": gc,
        "bec": bec,
        "eye": np.eye(32, dtype=f),
        "eye8h": np.eye(BPC, dtype=f),
        "mask": np.repeat(np.eye(BPC, dtype=f), C, axis=1),
        "mask4": np.tile(np.eye(C, dtype=f), (BPC, 1)),
        "ones1": np.ones((1, 128), dtype=f),
        "eps": np.full((40, 1), LN_EPS, dtype=f),
    }
    in_maps = []
    for m in range(N_CORES):
        sl = slice(m * BPC, (m + 1) * BPC)
        blocks = dict(const_blocks)
        blocks["xq"] = q[sl, :, 0].T
        blocks["xk"] = np.transpose(k[sl, :, 0, :], (1, 0, 2)).reshape(
            DIM, C * BPC
        )
        blocks["mdp"] = md[sl] * np.float32(MD_SCALE)
        in_maps.append({
            "v": np.ascontiguousarray(v[sl]).reshape(BPC * ROWS, FW),
            "params": _pack_params(blocks),
        })
    return in_maps


def _run(in_maps, **kwargs):
    nc = _get_nc()
    return run_bass_kernel_spmd(nc, in_maps, core_ids=list(range(N_CORES)), **kwargs)


def _gather(res):
    out_full = np.empty((B, V_CH, IMG, IMG), np.float32)
    attn_full = np.empty((B, C), np.float32)
    for m in range(N_CORES):
        r = res.results[m]
        out_full[m * BPC : (m + 1) * BPC] = r["out"].reshape(BPC, V_CH, IMG, IMG)
        attn_full[m * BPC : (m + 1) * BPC] = r["attn"]
    attn_map = np.ascontiguousarray(
        np.broadcast_to(attn_full[:, :, None, None], (B, C, IMG, IMG))
    )
    return out_full, attn_map


def kernel(**inputs):
    in_maps = _make_in_maps(**inputs)
    res = _run(in_maps)
    return _gather(res)


def kernel_profiled(**inputs):
    """Like kernel(), but also returns BassKernelResults with trace info."""
    in_maps = _make_in_maps(**inputs)
    res = _run(in_maps, trace=True)
    return _gather(res), res
